# revision 27
# baseline (speedup 1.0000x reference)
"""Distributed memory-shard scale kernel for Trainium2 (8 NeuronCores).

Computes out[b, s, d] = x[b, s, d] * shards[shard_map[d], d] for
x: [4, 4096, 4096] f32, shards: [8, 4096] f32, shard_map: [4096] int.

Strategy: data-parallel over the flattened (batch*seq) rows — each of the
8 cores owns a contiguous 2048-row slice of x and replicates the tiny
shards/shard_map inputs. On device each core:
  1. builds w[d] = shards[shard_map[d], d] with masked multiply-accumulate
     over the 8 shard rows, 256 dims per partition on 16 partitions,
  2. flattens w onto one partition with a single SBUF→SBUF DMA and
     replicates it to all 128 partitions with K=1 outer-product matmuls
     (PE + DVE only — no broadcast DMA traffic),
  3. streams its x slice through SBUF in [128, 4096] tiles, multiplying by
     the replicated weight row and writing back out. The first row block
     is processed in quarter-width chunks so the store stream starts as
     early as possible.
"""

import numpy as np

import bass_rust as _bass_rust
import concourse.bass as bass
import concourse.tile as tile
from concourse import mybir
from concourse.bass_utils import run_bass_kernel_spmd

N_CORES = 8
BATCH, SEQ, DIM = 4, 4096, 4096
NUM_SHARDS = 8
ROWS_TOTAL = BATCH * SEQ               # 16384
ROWS_PER_CORE = ROWS_TOTAL // N_CORES  # 2048
P = 128                                # SBUF partitions
N_TILES = ROWS_PER_CORE // P           # 16
WP = 16                                # partitions used by the w build
DPW = DIM // WP                        # dims per partition in w build (256)
AUX_W = (1 + NUM_SHARDS) * DPW         # aux free width (2304)

TRACE = False       # set True (e.g. from test.py) to capture an NTFF profile
LAST_RESULT = None  # BassKernelResults of the most recent kernel() call

_cached_nc = None


def _build_program() -> bass.Bass:
    f32 = mybir.dt.float32
    nc = bass.Bass()
    x_in = nc.dram_tensor("x", [ROWS_PER_CORE, DIM], f32, kind="ExternalInput")
    # aux packs shard_map and shards into one [16, 2304] tensor:
    #   aux[p, 0:DPW]         = shard_map[p*DPW : (p+1)*DPW]  (as f32)
    #   aux[p, (1+s)*DPW + j] = shards[s, p*DPW + j]
    aux_in = nc.dram_tensor("aux", [WP, AUX_W], f32, kind="ExternalInput")
    out = nc.dram_tensor("out", [ROWS_PER_CORE, DIM], f32,
                         kind="ExternalOutput")

    with tile.TileContext(nc) as tc:
        with tc.tile_pool(name="const", bufs=1) as cpool, \
             tc.tile_pool(name="xp", bufs=10) as xpool:
            # ones row for the broadcast matmuls — engine op, no DMA
            ones = cpool.tile([1, P], f32)
            nc.vector.memset(ones[:], 1.0)
            # --- one-time: w[d] = shards[shard_map[d], d], [16, 256] ---
            auxt = cpool.tile([WP, AUX_W], f32)
            nc.sync.dma_start(auxt[:], aux_in[:])
            mf = auxt[:, 0:DPW]
            wacc = cpool.tile([WP, DPW], f32)
            tmp = cpool.tile([WP, DPW], f32)
            nc.vector.memset(wacc[:], 0.0)
            for s in range(NUM_SHARDS):
                # tmp = (shard_map == s) * shards[s, :]
                nc.vector.scalar_tensor_tensor(
                    out=tmp[:], in0=mf, scalar=float(s),
                    in1=auxt[:, (1 + s) * DPW:(2 + s) * DPW],
                    op0=mybir.AluOpType.is_equal, op1=mybir.AluOpType.mult)
                nc.vector.tensor_add(wacc[:], wacc[:], tmp[:])

            # --- flatten w onto one partition (single SB→SB DMA on the
            # idle ACT ring), then replicate to all 128 partitions with
            # K=1 outer-product matmuls ones[1,128].T @ wrow[1,512] →
            # PSUM[128,512]; PE+DVE only.
            w128 = cpool.tile([P, DIM], f32)
            wrow = cpool.tile([1, DIM], f32)
            nc.scalar.dma_start(wrow[:], wacc[:])
            MMF = 512  # one PSUM bank per matmul
            with tc.tile_pool(name="ps", bufs=8, space="PSUM") as ppool:
                for k in range(DIM // MMF):
                    mm = ppool.tile([P, MMF], f32)
                    nc.tensor.matmul(mm[:], ones[:],
                                     wrow[0:1, k * MMF:(k + 1) * MMF],
                                     start=True, stop=True)
                    nc.vector.tensor_copy(w128[:, k * MMF:(k + 1) * MMF],
                                          mm[:])

            # --- stream x through SBUF, scaling by w ---
            # First row block in quarter-width chunks (its loads clear the
            # shared DMA-completion lanes fast, and the first stores issue
            # as soon as w is ready); last block split in half to shorten
            # the final mul→store chain.
            def chunks_for(i):
                if i == 0:
                    return 4
                if i == N_TILES - 1:
                    return 2
                return 1

            for i in range(N_TILES):
                n_ch = chunks_for(i)
                w_ch = DIM // n_ch
                xt = xpool.tile([P, DIM], f32)
                rows = slice(i * P, (i + 1) * P)
                for c in range(n_ch):
                    cols = slice(c * w_ch, (c + 1) * w_ch)
                    nc.sync.dma_start(xt[:, cols], x_in[rows, cols])
                    nc.vector.tensor_mul(xt[:, cols], xt[:, cols],
                                         w128[:, cols])
                    nc.scalar.dma_start(out[rows, cols], xt[:, cols])
    # TRN2 allows one sync wait per instruction; split multi-wait
    # instructions the way bacc's compile pipeline does.
    _bass_rust.generate_event_semaphores(nc)
    return nc


def _marshal(shards: np.ndarray, shard_map: np.ndarray):
    sh = np.asarray(shards, dtype=np.float32)
    aux = np.empty((WP, AUX_W), dtype=np.float32)
    aux[:, 0:DPW] = np.asarray(shard_map).astype(np.float32).reshape(WP, DPW)
    # aux[p, (1+s)*DPW + j] = shards[s, p*DPW + j]
    aux[:, DPW:] = sh.reshape(NUM_SHARDS, WP, DPW).transpose(
        1, 0, 2).reshape(WP, NUM_SHARDS * DPW)
    return aux


def kernel(x, shards, shard_map):
    global _cached_nc, LAST_RESULT
    if _cached_nc is None:
        _cached_nc = _build_program()
    nc = _cached_nc

    x2 = np.asarray(x, dtype=np.float32).reshape(ROWS_TOTAL, DIM)
    aux = _marshal(shards, shard_map)

    in_maps = [
        {"x": x2[c * ROWS_PER_CORE:(c + 1) * ROWS_PER_CORE], "aux": aux}
        for c in range(N_CORES)
    ]
    res = run_bass_kernel_spmd(nc, in_maps, core_ids=list(range(N_CORES)),
                               trace=TRACE)
    LAST_RESULT = res
    return np.concatenate([r["out"] for r in res.results],
                          axis=0).reshape(BATCH, SEQ, DIM)
